# revision 4
# baseline (speedup 1.0000x reference)
"""Multi-head causal attention with RoPE on 8 trn2 NeuronCores.

Problem (hardcoded): B=2, S=2048, D=2048, H=16, Hd=128, fp32.
  q/k/v = x @ wq/wk/wv; RoPE(q,k); causal softmax(q k^T/sqrt(Hd)) @ v; out @ wo.

Sharding: core c = 4*b + g handles batch b, heads [4g, 4g+4).
  - wq/wk/wv column-parallel (512 cols per core), wo column-parallel with an
    AllGather of the per-core attention outputs o^T inside each batch group
    of 4 cores (replaces the row-parallel all-reduce; 4MB gather instead of
    a 16MB reduce).
  - Host-side prep: per-head even/odd column permutation of wq/wk so RoPE
    becomes partition-aligned in the transposed [head_dim, S] layout.
  - Output: core (b,g) returns y = o_full_b @ wo[:, 512g:512(g+1)];
    host concatenates columns per batch.

All matmuls run as float32r (1 cycle/row on trn2 for moving dim >= 256).
"""
import math
import numpy as np

import concourse.bass as bass
import concourse.tile as tile
from concourse import bacc, mybir
from concourse.bass_utils import run_bass_kernel_spmd

F32 = mybir.dt.float32
F32R = mybir.dt.float32r
EXPF = mybir.ActivationFunctionType.Exp
ADD = mybir.AluOpType.add
MULT = mybir.AluOpType.mult

B, S, D = 2, 2048, 2048
H, HD = 16, 128
HPC = 4              # heads per core
DC = HPC * HD        # 512 d_out per core
NCHUNK = D // 128    # 16 contraction chunks
SB = 512             # phase-1 s-block
NSB = S // SB        # 4
QB = 512             # phase-2 q-block
NQB = S // QB        # 4
SCALE = 1.0 / math.sqrt(HD)
NEG = -1.0e30


def build_module():
    nc = bacc.Bacc("TRN2", target_bir_lowering=False, debug=False, num_devices=8)

    x = nc.dram_tensor("x", [S, D], F32R, kind="ExternalInput").ap()
    wq = nc.dram_tensor("wq", [D, DC], F32R, kind="ExternalInput").ap()
    wk = nc.dram_tensor("wk", [D, DC], F32R, kind="ExternalInput").ap()
    wv = nc.dram_tensor("wv", [D, DC], F32R, kind="ExternalInput").ap()
    wo = nc.dram_tensor("wo", [D, DC], F32R, kind="ExternalInput").ap()
    c2 = nc.dram_tensor("c2", [128, S], F32, kind="ExternalInput").ap()
    s2n = nc.dram_tensor("s2n", [128, S], F32, kind="ExternalInput").ap()
    tri = nc.dram_tensor("tri", [128, 128], F32, kind="ExternalInput").ap()
    ones = nc.dram_tensor("ones", [128, 128], F32R, kind="ExternalInput").ap()
    ident = nc.dram_tensor("ident", [128, 128], F32R, kind="ExternalInput").ap()
    y = nc.dram_tensor("y", [S, DC], F32, kind="ExternalOutput").ap()

    v_dram = nc.dram_tensor("v_dram", [S, DC], F32R)
    ot_loc = nc.dram_tensor("ot_loc", [DC, S], F32R)
    ot_full = nc.dram_tensor("ot_full", [D, S], F32R)

    with tile.TileContext(nc) as tc:
        with tc.tile_pool(name="consts", bufs=1) as cpool, \
             tc.tile_pool(name="qkres", bufs=1) as qkpool:
            ones_t = cpool.tile([128, 128], F32R)
            nc.sync.dma_start(ones_t[:], ones[:])
            tri_t = cpool.tile([128, 128], F32)
            nc.sync.dma_start(tri_t[:], tri[:])
            id_t = cpool.tile([128, 128], F32R)
            nc.sync.dma_start(id_t[:], ident[:])
            c2_t = cpool.tile([128, S], F32)
            nc.sync.dma_start(c2_t[:], c2[:])
            s2n_t = cpool.tile([128, S], F32)
            nc.sync.dma_start(s2n_t[:], s2n[:])

            # persistent q^T/k^T per head: [128, S]
            qt_res = [qkpool.tile([128, S], F32R, name=f"qt{h}") for h in range(HPC)]
            kt_res = [qkpool.tile([128, S], F32R, name=f"kt{h}") for h in range(HPC)]

            # ---------------- Phase 1: projections + RoPE ----------------
            with tc.tile_pool(name="p1sb", bufs=2) as p1, \
                 tc.tile_pool(name="p1xt", bufs=1) as p1x, \
                 tc.tile_pool(name="p1ps", bufs=2, space="PSUM") as p1ps:
                for j in range(NSB):
                    s0 = j * SB
                    # transpose x[s0:s0+SB, :] -> xT chunks [128, SB]
                    xt = [p1x.tile([128, SB], F32R, name=f"xt{c}", tag=f"xt{c}")
                          for c in range(NCHUNK)]
                    for ss in range(SB // 128):
                        xrow = p1.tile([128, D], F32R, tag="xrow", bufs=3)
                        nc.sync.dma_start(xrow[:], x[s0 + ss * 128:s0 + (ss + 1) * 128, :])
                        for c4 in range(NCHUNK // 4):
                            tp = p1ps.tile([128, 512], F32R, tag="tpps")
                            for cc in range(4):
                                c = c4 * 4 + cc
                                nc.tensor.transpose(
                                    tp[:, cc * 128:(cc + 1) * 128],
                                    xrow[:, c * 128:(c + 1) * 128], id_t[:])
                            for cc in range(4):
                                c = c4 * 4 + cc
                                nc.scalar.copy(
                                    xt[c][:, ss * 128:(ss + 1) * 128],
                                    tp[:, cc * 128:(cc + 1) * 128])

                    # q-pass then k-pass: chunk-outer streaming of weights,
                    # 4 PSUM accumulators (one per head) held per pass.
                    for (wsrc, res_list, wtag) in ((wq, qt_res, "wq"),
                                                   (wk, kt_res, "wk")):
                        prj = [p1ps.tile([128, SB], F32, tag=f"acc{h}", bufs=1,
                                         name=f"prj{h}") for h in range(HPC)]
                        for c in range(NCHUNK):
                            wt = p1.tile([128, DC], F32R, tag=wtag, bufs=3,
                                         name=f"{wtag}t{c}")
                            nc.sync.dma_start(wt[:], wsrc[c * 128:(c + 1) * 128, :])
                            for h in range(HPC):
                                nc.tensor.matmul(
                                    prj[h][:], wt[:, h * 128:(h + 1) * 128],
                                    xt[c][:], start=(c == 0), stop=(c == NCHUNK - 1))
                        for h in range(HPC):
                            raw = p1.tile([128, SB], F32, tag="rraw")
                            nc.scalar.copy(raw[:], prj[h][:])
                            swp = p1.tile([128, SB], F32, tag="rswp")
                            nc.sync.dma_start(swp[0:64, :], raw[64:128, :])
                            nc.sync.dma_start(swp[64:128, :], raw[0:64, :])
                            t1 = p1.tile([128, SB], F32, tag="rt1")
                            nc.vector.tensor_tensor(
                                t1[:], prj[h][:], c2_t[:, s0:s0 + SB], op=MULT)
                            t2 = p1.tile([128, SB], F32, tag="rt2")
                            nc.vector.tensor_tensor(
                                t2[:], swp[:], s2n_t[:, s0:s0 + SB], op=MULT)
                            nc.vector.tensor_tensor(
                                res_list[h][:, s0:s0 + SB], t1[:], t2[:], op=ADD)

                    # v-pass (natural layout), chunk-outer, spill to DRAM
                    vps = [p1ps.tile([128, DC], F32, tag=f"acc{ss}", bufs=1,
                                     name=f"vps{ss}") for ss in range(SB // 128)]
                    for c in range(NCHUNK):
                        wt = p1.tile([128, DC], F32R, tag="wv", bufs=3,
                                     name=f"wvt{c}")
                        nc.sync.dma_start(wt[:], wv[c * 128:(c + 1) * 128, :])
                        for ss in range(SB // 128):
                            nc.tensor.matmul(
                                vps[ss][:], xt[c][:, ss * 128:(ss + 1) * 128],
                                wt[:], start=(c == 0), stop=(c == NCHUNK - 1))
                    for ss in range(SB // 128):
                        vsb = p1.tile([128, DC], F32R, tag="vsb")
                        nc.scalar.copy(vsb[:], vps[ss][:])
                        nc.sync.dma_start(
                            v_dram[s0 + ss * 128:s0 + (ss + 1) * 128, :], vsb[:])

            # ---------------- Phase 2: attention ----------------
            with tc.tile_pool(name="p2v", bufs=1) as p2v, \
                 tc.tile_pool(name="p2sb", bufs=3) as p2, \
                 tc.tile_pool(name="p2acc", bufs=2, space="PSUM") as p2acc, \
                 tc.tile_pool(name="p2sc", bufs=3, space="PSUM") as p2sc:
                v_t = [p2v.tile([128, DC], F32R, name=f"v{kb}") for kb in range(S // 128)]
                for kb in range(S // 128):
                    nc.sync.dma_start(v_t[kb][:], v_dram[kb * 128:(kb + 1) * 128, :])

                for h in range(HPC):
                    for j in range(NQB):
                        q0 = j * QB
                        nkb = 4 * (j + 1)
                        pv = p2acc.tile([128, QB], F32, tag="pv")
                        dn = p2acc.tile([128, QB], F32, tag="dn")
                        for kb in range(nkb):
                            r = kb - 4 * j
                            if r < 0:
                                lo = 0
                            elif r <= 2:
                                lo = r * 128
                            else:
                                lo = 256
                            w = QB - lo
                            sc = p2sc.tile([128, QB], F32, tag="sc")
                            nc.tensor.matmul(
                                sc[:, lo:], kt_res[h][:, kb * 128:(kb + 1) * 128],
                                qt_res[h][:, q0 + lo:q0 + QB],
                                start=True, stop=True)
                            if r >= 0:
                                nc.vector.tensor_tensor(
                                    sc[:, r * 128:(r + 1) * 128],
                                    sc[:, r * 128:(r + 1) * 128], tri_t[:], op=ADD)
                            if r == 3:
                                # widened dead zone: force exp() to 0 there
                                nc.vector.tensor_scalar_add(
                                    sc[:, 256:384], sc[:, 256:384], NEG)
                            ep = p2.tile([128, QB], F32R, tag="ep")
                            nc.scalar.activation(ep[:, lo:], sc[:, lo:], EXPF,
                                                 scale=SCALE)
                            nc.tensor.matmul(
                                dn[:, lo:], ones_t[:], ep[:, lo:],
                                start=(kb == 0), stop=(kb == nkb - 1),
                                skip_group_check=True)
                            nc.tensor.matmul(
                                pv[:, lo:], v_t[kb][:, h * 128:(h + 1) * 128],
                                ep[:, lo:],
                                start=(kb == 0), stop=(kb == nkb - 1),
                                skip_group_check=True)
                        rec = p2.tile([128, QB], F32, tag="rec")
                        nc.vector.reciprocal(rec[:], dn[:])
                        ot = p2.tile([128, QB], F32R, tag="ot")
                        nc.vector.tensor_tensor(ot[:], pv[:], rec[:], op=MULT)
                        nc.sync.dma_start(
                            ot_loc[h * 128:(h + 1) * 128, q0:q0 + QB], ot[:])

            # AllGather o^T within each batch group of 4
            nc.gpsimd.collective_compute(
                "AllGather",
                mybir.AluOpType.bypass,
                replica_groups=[[0, 1, 2, 3], [4, 5, 6, 7]],
                ins=[ot_loc[:]],
                outs=[ot_full[:]],
            )

            # ---------------- Phase 3: output projection ----------------
            with tc.tile_pool(name="p3wo", bufs=1) as p3w, \
                 tc.tile_pool(name="p3sb", bufs=3) as p3, \
                 tc.tile_pool(name="p3ps", bufs=2, space="PSUM") as p3ps:
                wo_t = [p3w.tile([128, DC], F32R, name=f"wo{c}") for c in range(NCHUNK)]
                for c in range(NCHUNK):
                    nc.sync.dma_start(wo_t[c][:], wo[c * 128:(c + 1) * 128, :])
                for sq in range(4):
                    o0 = sq * 512
                    otf = [p3.tile([128, 512], F32R, tag=f"otf{c}", bufs=2,
                                   name=f"otf{c}_{sq}") for c in range(NCHUNK)]
                    for c in range(NCHUNK):
                        nc.sync.dma_start(
                            otf[c][:], ot_full[c * 128:(c + 1) * 128, o0:o0 + 512])
                    for ss in range(4):
                        yps = p3ps.tile([128, DC], F32, tag="yps")
                        for c in range(NCHUNK):
                            nc.tensor.matmul(
                                yps[:], otf[c][:, ss * 128:(ss + 1) * 128],
                                wo_t[c][:], start=(c == 0), stop=(c == NCHUNK - 1))
                        ysb = p3.tile([128, DC], F32, tag="ysb")
                        nc.scalar.copy(ysb[:], yps[:])
                        nc.sync.dma_start(
                            y[o0 + ss * 128:o0 + (ss + 1) * 128, :], ysb[:])

    nc.compile()
    return nc


_PERM = np.concatenate([np.arange(0, 128, 2), np.arange(1, 128, 2)])


def make_in_maps(x, wq, wk, wv, wo, freqs_cos, freqs_sin):
    """Host-side sharding/prep. Returns list of 8 per-core input dicts."""
    cosT = np.ascontiguousarray(freqs_cos.T.astype(np.float32))   # [64, S]
    sinT = np.ascontiguousarray(freqs_sin.T.astype(np.float32))
    c2 = np.concatenate([cosT, cosT], axis=0)                     # [128, S]
    s2n = np.concatenate([-sinT, sinT], axis=0)
    tri = np.where(np.arange(128)[None, :] >= np.arange(128)[:, None],
                   0.0, NEG).astype(np.float32)                   # [k, q]
    ones = np.ones((128, 128), dtype=np.float32)
    ident = np.eye(128, dtype=np.float32)

    in_maps = []
    for c in range(8):
        b, g = divmod(c, 4)
        cols = slice(g * DC, (g + 1) * DC)
        wq_c = np.ascontiguousarray(wq[:, cols]).copy()
        wk_c = np.ascontiguousarray(wk[:, cols]).copy()
        for h in range(HPC):
            blk = slice(h * 128, (h + 1) * 128)
            wq_c[:, blk] = wq_c[:, blk][:, _PERM]
            wk_c[:, blk] = wk_c[:, blk][:, _PERM]
        in_maps.append({
            "x": np.ascontiguousarray(x[b]).astype(np.float32),
            "wq": wq_c.astype(np.float32),
            "wk": wk_c.astype(np.float32),
            "wv": np.ascontiguousarray(wv[:, cols]).astype(np.float32),
            "wo": np.ascontiguousarray(wo[:, cols]).astype(np.float32),
            "c2": c2, "s2n": s2n, "tri": tri, "ones": ones, "ident": ident,
        })
    return in_maps


def assemble(results):
    """Concatenate per-core column outputs into [B, S, D]."""
    out = np.empty((B, S, D), dtype=np.float32)
    for c in range(8):
        b, g = divmod(c, 4)
        out[b][:, g * DC:(g + 1) * DC] = results[c]["y"]
    return out


_NC = None


def kernel(x, wq, wk, wv, wo, freqs_cos, freqs_sin):
    global _NC
    x = np.asarray(x); wq = np.asarray(wq); wk = np.asarray(wk)
    wv = np.asarray(wv); wo = np.asarray(wo)
    freqs_cos = np.asarray(freqs_cos); freqs_sin = np.asarray(freqs_sin)
    if _NC is None:
        _NC = build_module()
    in_maps = make_in_maps(x, wq, wk, wv, wo, freqs_cos, freqs_sin)
    res = run_bass_kernel_spmd(_NC, in_maps, core_ids=list(range(8)))
    return assemble(res.results)
